# revision 33
# baseline (speedup 1.0000x reference)
"""Trainium2 Bass kernel for DenseEquiMessageBlock (B=2, N=384, F=128).

Sharding: receiver-axis (j) sharding. Core c -> batch b=c//4, receiver block
js=(c%4)*96. Every sum-over-senders contraction is local to a core, so no
collectives are needed; per-core outputs are disjoint slices of dh/dv.

Wait discipline: this toolchain's walrus encodes at most ONE sync-wait per
compute instruction, so the kernel is structured so every instruction has at
most one unobserved producer: an observer gadget makes each engine see every
input-DMA queue once; each PSUM tile is read by exactly one engine; pool bufs
are sized so slot-recycle waits are elided by ticks each engine already
observed; the rbf stage computes (d-off)^2 transposed per sender block (d as
per-partition ACT bias) and PE-transposes back, avoiding per-row DMAs.
"""

import numpy as np

import concourse.bass as bass
import concourse.mybir as mybir
from concourse.tile import TileContext
from concourse.bass_utils import run_bass_kernel_spmd

AF = mybir.ActivationFunctionType
ALU = mybir.AluOpType
AX = mybir.AxisListType
DT = mybir.dt.float32

B, N, F = 2, 384, 128
NC = 8
JB = N // 4          # receivers per core
NIB = N // 128       # sender blocks of 128
LN_EPS = 1e-5
RECIP_GUARD = 1e-12


def build_program() -> bass.Bass:
    nc = bass.Bass()

    dp = lambda name, shape: nc.declare_dram_parameter(name, list(shape), DT, isOutput=False)
    hT_d = dp("hT", (F, N))
    hTj_d = dp("hTj", (F, JB))
    xyzT_d = dp("xyzT", (3, N))
    xyzTj_d = dp("xyzTj", (3, JB))
    xyzjd_d = dp("xyzjd", (1, JB * 3))
    xyzr_d = dp("xyzr", (N, 3))
    adjp_d = dp("adjp", (NIB, 128, JB))
    vp_d = dp("vp", (NIB, 128, 3, F))
    W_d = [dp(f"W{k}", (F, F)) for k in (1, 2, 3, 4, 5)]
    W6p_d = dp("W6p", (F, 3 * F))
    b_d = [dp(f"b{k}", (F, 1)) for k in (1, 2, 3, 4, 5)]
    b6r_d = dp("b6r", (1, F))
    b6v_d = dp("b6v", (F, 1))
    b6h_d = dp("b6h", (1, F))
    gamma_d = dp("gamma", (1, F))
    beta_d = dp("beta", (1, F))
    offr_d = dp("offr", (1, F))
    ident_d = dp("ident", (128, 128))

    dh_out_d = nc.declare_dram_parameter("dh_out", [JB, F], DT, isOutput=True)
    dv_out_d = nc.declare_dram_parameter("dv_out", [F, JB * 3], DT, isOutput=True)

    with TileContext(nc) as tc:
        with (
            tc.tile_pool(name="const", bufs=1) as cpool,
            tc.tile_pool(name="work", bufs=2) as wpool,
            tc.tile_pool(name="red", bufs=2) as rpool,
            tc.tile_pool(name="ps_dh", bufs=1, space="PSUM") as ps_dh,
        ):
            # ---------------- constants in SBUF -----------------------
            ones = cpool.tile([128, N], DT, tag="ones")
            nc.vector.memset(ones[:], 1.0)
            ident = cpool.tile([128, 128], DT, tag="ident")
            nc.sync.dma_start(out=ident[:], in_=ident_d[:])
            hT = cpool.tile([F, N], DT, tag="hT")
            nc.sync.dma_start(out=hT[:], in_=hT_d[:])
            hTj = cpool.tile([F, JB], DT, tag="hTj")
            nc.sync.dma_start(out=hTj[:], in_=hTj_d[:])
            xyzT = cpool.tile([3, N], DT, tag="xyzT")
            nc.sync.dma_start(out=xyzT[:], in_=xyzT_d[:])
            xyzTj = cpool.tile([3, JB], DT, tag="xyzTj")
            nc.sync.dma_start(out=xyzTj[:], in_=xyzTj_d[:])
            xyzjd = cpool.tile([1, JB * 3], DT, tag="xyzjd")
            nc.sync.dma_start(out=xyzjd[:], in_=xyzjd_d[:])
            xyzib = cpool.tile([128, 3 * NIB], DT, tag="xyzib")
            for ib in range(NIB):
                nc.sync.dma_start(
                    out=xyzib[:, ib * 3:(ib + 1) * 3],
                    in_=xyzr_d[ib * 128:(ib + 1) * 128, :],
                )
            adjs = cpool.tile([128, NIB * JB], DT, tag="adjs")  # col = ib*JB + j
            nc.sync.dma_start(
                out=adjs[:].rearrange("p (b j) -> p b j", b=NIB),
                in_=adjp_d[:].transpose([1, 0, 2]),
            )
            vsb = cpool.tile([128, NIB * 3 * F], DT, tag="vsb")  # col = ib*384+d*128+f
            nc.sync.dma_start(
                out=vsb[:].rearrange("p (b d f) -> p b d f", b=NIB, d=3),
                in_=vp_d[:].transpose([1, 0, 2, 3]),
            )
            Ws = []
            for k in range(5):
                t = cpool.tile([F, F], DT, tag=f"W{k + 1}")
                nc.sync.dma_start(out=t[:], in_=W_d[k][:])
                Ws.append(t)
            W6p = cpool.tile([F, 3 * F], DT, tag="W6p")
            nc.sync.dma_start(out=W6p[:], in_=W6p_d[:])
            bs = []
            for k in range(5):
                t = cpool.tile([F, 1], DT, tag=f"b{k + 1}")
                nc.sync.dma_start(out=t[:], in_=b_d[k][:])
                bs.append(t)
            b6r = cpool.tile([1, F], DT, tag="b6r")
            nc.sync.dma_start(out=b6r[:], in_=b6r_d[:])
            b6v = cpool.tile([F, 1], DT, tag="b6v")
            nc.sync.dma_start(out=b6v[:], in_=b6v_d[:])
            b6h = cpool.tile([1, F], DT, tag="b6h")
            nc.sync.dma_start(out=b6h[:], in_=b6h_d[:])
            gamma_r = cpool.tile([1, F], DT, tag="gamma_r")
            nc.sync.dma_start(out=gamma_r[:], in_=gamma_d[:])
            beta_r = cpool.tile([1, F], DT, tag="beta_r")
            nc.sync.dma_start(out=beta_r[:], in_=beta_d[:])
            offr = cpool.tile([1, F], DT, tag="offr")
            nc.sync.dma_start(out=offr[:], in_=offr_d[:])

            xyzTj_m2 = cpool.tile([3, JB], DT, tag="xyzTj_m2")
            nc.vector.tensor_scalar_mul(xyzTj_m2[:], xyzTj[:], -2.0)
            eps_col = cpool.tile([JB, 1], DT, tag="eps_col")
            nc.vector.memset(eps_col[:], LN_EPS)

            d_rows = cpool.tile([JB, N], DT, tag="d_rows")
            negd = cpool.tile([128, NIB * JB], DT, tag="negd")
            racp = cpool.tile([128, NIB * JB], DT, tag="racp")
            unitA = cpool.tile([128, NIB * JB * 3], DT, tag="unitA")
            s_uA = cpool.tile([1, JB * 3], DT, tag="s_uA")
            adjrowsum = cpool.tile([1, JB], DT, tag="adjrowsum")
            bv_vadj = cpool.tile([F, 3 * JB], DT, tag="bv_vadj")
            phi = cpool.tile([F, N], DT, tag="phi")
            phi_r = cpool.tile([F, JB], DT, tag="phi_r")
            gamma_bc = cpool.tile([JB, F], DT, tag="gamma_bc")
            beta_bc = cpool.tile([JB, F], DT, tag="beta_bc")
            off_bc = cpool.tile([128, F], DT, tag="off_bc")
            dv_sb = cpool.tile([F, JB * 3], DT, tag="dv_sb")

            dh_pre = ps_dh.tile([F, JB], DT, tag="dh_pre")
            # dv accumulator [f, (j,d)]: one eternal PSUM accumulation group
            # (re-opening groups on recycled banks costs unencodable PE waits)
            dv_ps = ps_dh.tile([F, JB * 3], DT, tag="dv_ps")

            # ================= setup phase ============================
            with tc.tile_pool(name="sps", bufs=5, space="PSUM") as sps:
                # Observer gadget: every engine sees each input-DMA queue once
                # so later instructions carry at most one fresh wait.
                obs_slices = [ident[:, 0:1], hT[:, 0:1], hTj[:, 0:1], xyzT[:, 0:1],
                              xyzTj[:, 0:1], xyzjd[:, 0:1], adjs[:, 0:1], vsb[:, 0:1]]
                obs_slices += [xyzib[:, ib * 3:ib * 3 + 1] for ib in range(NIB)]
                obs_slices += [t[:, 0:1] for t in Ws] + [W6p[:, 0:1]]
                obs_slices += [t[:, 0:1] for t in bs]
                obs_slices += [b6r[:, 0:1], b6v[:, 0:1], b6h[:, 0:1],
                               gamma_r[:, 0:1], beta_r[:, 0:1], offr[:, 0:1]]
                nobs = len(obs_slices)
                obs_ps = sps.tile([1, nobs + 1], DT, tag="obs", bufs=1)
                obs_dve = wpool.tile([128, nobs], DT, tag="obs_dve", bufs=1)
                obs_act = wpool.tile([128, nobs], DT, tag="obs_act", bufs=1)
                nc.tensor.matmul(obs_ps[:, 0:1], ones[:1, 0:1], ones[:1, 0:1],
                                 start=True, stop=True)
                for c, sl in enumerate(obs_slices):
                    kp = sl.shape[0]
                    nc.tensor.matmul(obs_ps[:, c + 1:c + 2], ones[:kp, 0:1], sl,
                                     start=True, stop=True)
                    nc.vector.tensor_copy(obs_dve[:kp, c:c + 1], sl)
                    nc.scalar.activation(obs_act[:kp, c:c + 1], sl, AF.Copy)

                # normsq rows |xyz|^2 (senders and local receivers)
                sq = wpool.tile([3, N], DT, tag="sq3")
                nc.scalar.activation(sq[:], xyzT[:], AF.Square)
                ps_nrm = sps.tile([1, N], DT, tag="sps")
                nc.tensor.matmul(ps_nrm[:], ones[:3, 0:1], sq[:], start=True, stop=True)
                normsq = cpool.tile([1, N], DT, tag="normsq")
                nc.scalar.activation(normsq[:], ps_nrm[:], AF.Copy)
                sqj = wpool.tile([3, JB], DT, tag="sqj")
                nc.scalar.activation(sqj[:], xyzTj[:], AF.Square)
                ps_nrj = sps.tile([1, JB], DT, tag="sps")
                nc.tensor.matmul(ps_nrj[:], ones[:3, 0:1], sqj[:], start=True, stop=True)
                normsqj = cpool.tile([1, JB], DT, tag="normsqj")
                nc.scalar.activation(normsqj[:], ps_nrj[:], AF.Copy)

                # d_rows[j, i] = |xyz_{js+j} - xyz_i|
                ps_d2 = sps.tile([JB, N], DT, tag="sps")
                nc.tensor.matmul(ps_d2[:], xyzTj_m2[:], xyzT[:], start=True, stop=False)
                nc.tensor.matmul(ps_d2[:], normsqj[:], ones[:1, :], start=False, stop=False)
                nc.tensor.matmul(ps_d2[:], ones[:1, :JB], normsq[:], start=False, stop=True)
                d2r = wpool.tile([JB, N], DT, tag="d2r")
                nc.vector.tensor_scalar_max(d2r[:], ps_d2[:], 0.0)
                nc.scalar.activation(d_rows[:], d2r[:], AF.Sqrt)

                # d columns (negated) + racp[i,(ib,j)] = adj/(d + guard)
                for ib in range(NIB):
                    ps_t = sps.tile([128, JB], DT, tag="sps")
                    nc.tensor.transpose(
                        ps_t[:], d_rows[:, ib * 128:(ib + 1) * 128], ident[:JB, :JB]
                    )
                    nc.vector.tensor_scalar_mul(
                        negd[:, ib * JB:(ib + 1) * JB], ps_t[:], -1.0
                    )
                    dcol = wpool.tile([128, JB], DT, tag="dcol")
                    nc.vector.tensor_scalar_add(dcol[:], ps_t[:], RECIP_GUARD)
                    rec = wpool.tile([128, JB], DT, tag="rec")
                    nc.vector.reciprocal(rec[:], dcol[:])
                    nc.vector.tensor_mul(
                        racp[:, ib * JB:(ib + 1) * JB], rec[:],
                        adjs[:, ib * JB:(ib + 1) * JB],
                    )

                # unitA[i, (j,d)] per ib = (xyz_j - xyz_i) * racp
                for ib in range(NIB):
                    ps_u = sps.tile([128, JB * 3], DT, tag="sps")
                    nc.tensor.matmul(ps_u[:], ones[:1, :128], xyzjd[:], start=True, stop=True)
                    blk = unitA[:, ib * JB * 3:(ib + 1) * JB * 3].rearrange(
                        "p (j d) -> p j d", d=3
                    )
                    xyzrep = (
                        xyzib[:, ib * 3:(ib + 1) * 3].unsqueeze(1).to_broadcast((128, JB, 3))
                    )
                    tmp_u = wpool.tile([128, JB * 3], DT, tag="tmp_u")
                    tmp_v = tmp_u[:].rearrange("p (j d) -> p j d", d=3)
                    nc.vector.tensor_sub(
                        tmp_v, ps_u[:].rearrange("p (j d) -> p j d", d=3), xyzrep
                    )
                    racprep = (
                        racp[:, ib * JB:(ib + 1) * JB].unsqueeze(2).to_broadcast((128, JB, 3))
                    )
                    nc.vector.tensor_mul(blk, tmp_v, racprep)

                # s_uA[1, (j,d)] = sum_i unitA
                ps_sa = sps.tile([1, JB * 3], DT, tag="sps")
                for ib in range(NIB):
                    nc.tensor.matmul(
                        ps_sa[:], ones[:, 0:1], unitA[:, ib * JB * 3:(ib + 1) * JB * 3],
                        start=(ib == 0), stop=(ib == NIB - 1),
                    )
                nc.scalar.activation(s_uA[:], ps_sa[:], AF.Copy)

                # adjrowsum[1, j] = sum_i adj[i, j]
                ps_ar = sps.tile([1, JB], DT, tag="sps")
                for ib in range(NIB):
                    nc.tensor.matmul(
                        ps_ar[:], ones[:, 0:1], adjs[:, ib * JB:(ib + 1) * JB],
                        start=(ib == 0), stop=(ib == NIB - 1),
                    )
                nc.scalar.activation(adjrowsum[:], ps_ar[:], AF.Copy)

                # bv_vadj[f, (j,d)] = b6v_f * sum_i adj[i,j] v[i,f,d]
                for d in range(3):
                    ps_v = sps.tile([F, JB], DT, tag="sps")
                    for ib in range(NIB):
                        voff = ib * 3 * F + d * F
                        nc.tensor.matmul(
                            ps_v[:], vsb[:, voff:voff + F],
                            adjs[:, ib * JB:(ib + 1) * JB],
                            start=(ib == 0), stop=(ib == NIB - 1),
                        )
                    out_view = bv_vadj[:].rearrange("p (j d) -> p j d", d=3)[:, :, d]
                    nc.vector.tensor_scalar_mul(out_view, ps_v[:], b6v[:])

                # phi over all senders; phi_r over local receivers
                ps_p1 = sps.tile([F, N], DT, tag="sps")
                nc.tensor.matmul(ps_p1[:], Ws[0][:], hT[:], start=True, stop=True)
                u1 = wpool.tile([F, N], DT, tag="u1")
                nc.scalar.activation(u1[:], ps_p1[:], AF.Silu, bias=bs[0][:])
                ps_p2 = sps.tile([F, N], DT, tag="sps")
                nc.tensor.matmul(ps_p2[:], Ws[1][:], u1[:], start=True, stop=True)
                nc.vector.tensor_scalar_add(phi[:], ps_p2[:], bs[1][:])

                ps_q1 = sps.tile([F, JB], DT, tag="sps")
                nc.tensor.matmul(ps_q1[:], Ws[0][:], hTj[:], start=True, stop=True)
                u1r = wpool.tile([F, JB], DT, tag="u1r")
                nc.scalar.activation(u1r[:], ps_q1[:], AF.Silu, bias=bs[0][:])
                ps_q2 = sps.tile([F, JB], DT, tag="sps")
                nc.tensor.matmul(ps_q2[:], Ws[1][:], u1r[:], start=True, stop=True)
                nc.vector.tensor_scalar_add(phi_r[:], ps_q2[:], bs[1][:])

                # broadcast rows -> [JB, F] / [128, F]
                ps_g = sps.tile([JB, F], DT, tag="sps")
                nc.tensor.matmul(ps_g[:], ones[:1, :JB], gamma_r[:], start=True, stop=True)
                nc.scalar.activation(gamma_bc[:], ps_g[:], AF.Copy)
                ps_b = sps.tile([JB, F], DT, tag="sps")
                nc.tensor.matmul(ps_b[:], ones[:1, :JB], beta_r[:], start=True, stop=True)
                nc.scalar.activation(beta_bc[:], ps_b[:], AF.Copy)
                ps_o = sps.tile([128, F], DT, tag="sps")
                nc.tensor.matmul(ps_o[:], ones[:1, :128], offr[:], start=True, stop=True)
                nc.scalar.activation(off_bc[:], ps_o[:], AF.Copy)

                # dh_pre init: b6h x adjrowsum (the fh bias correction)
                nc.tensor.matmul(dh_pre[:], b6h[:], adjrowsum[:], start=True, stop=False,
                                 skip_group_check=True)
                # dv_ps init: the b6r rank-1 bias for ALL receivers at once
                nc.tensor.matmul(dv_ps[:], b6r[:], s_uA[:], start=True, stop=False,
                                 skip_group_check=True)
                # let ACT observe its own off_bc drain + DVE's negd before j=0
                nc.scalar.activation(obs_act[:, 0:1], off_bc[:, 0:1], AF.Copy)
                nc.scalar.activation(obs_act[:, 1:2], negd[:, 0:1], AF.Copy)

            # ================= main loop over receivers ===============
            with (
                tc.tile_pool(name="mps1", bufs=1, space="PSUM") as mps1,
                tc.tile_pool(name="mps2", bufs=2, space="PSUM") as mps2,
            ):
                for j in range(JB):
                    # rbf[f, i] = exp(-(d_j[i]-off_f)^2): build (off-d)^2
                    # transposed [i, f] with d as per-partition ACT bias,
                    # PE-transpose back to [f, i].
                    ps_tq = mps1.tile([F, N], DT, tag="tsq", bufs=1)
                    for ib in range(NIB):
                        tsqT = rpool.tile([128, F], DT, tag="tsqT", bufs=3)
                        nc.scalar.activation(
                            tsqT[:], off_bc[:], AF.Square,
                            bias=negd[:, ib * JB + j: ib * JB + j + 1],
                        )
                        nc.tensor.transpose(
                            ps_tq[:, ib * 128:(ib + 1) * 128], tsqT[:], ident[:]
                        )
                    rbf = wpool.tile([F, N], DT, tag="rbf")
                    nc.scalar.activation(rbf[:], ps_tq[:], AF.Exp, scale=-1.0)

                    # w = MLP2(rbf)
                    ps_u3 = mps1.tile([F, N], DT, tag="u3", bufs=1)
                    nc.tensor.matmul(ps_u3[:], Ws[2][:], rbf[:], start=True, stop=True)
                    u3 = wpool.tile([F, N], DT, tag="u3")
                    nc.scalar.activation(u3[:], ps_u3[:], AF.Silu, bias=bs[2][:])
                    ps_w = mps1.tile([F, N], DT, tag="w", bufs=1)
                    nc.tensor.matmul(ps_w[:], Ws[3][:], u3[:], start=True, stop=True)
                    wt = wpool.tile([F, N], DT, tag="wt")
                    nc.vector.tensor_scalar_add(wt[:], ps_w[:], bs[3][:])

                    # m = w * phi_j * phi_i ; g = silu(W5.T m + b5)
                    m = wpool.tile([F, N], DT, tag="m")
                    nc.vector.scalar_tensor_tensor(
                        m[:], wt[:], phi_r[:, j:j + 1], phi[:],
                        op0=ALU.mult, op1=ALU.mult,
                    )
                    ps_gm = mps1.tile([F, N], DT, tag="gm", bufs=1)
                    nc.tensor.matmul(ps_gm[:], Ws[4][:], m[:], start=True, stop=True)
                    g = wpool.tile([F, N], DT, tag="g")
                    nc.scalar.activation(g[:], ps_gm[:], AF.Silu, bias=bs[4][:])

                    for ib in range(NIB):
                        ps_fwt = mps2.tile([128, 3 * F], DT, tag="fwt")
                        nc.tensor.matmul(
                            ps_fwt[:], g[:, ib * 128:(ib + 1) * 128], W6p[:],
                            start=True, stop=True,
                        )
                        # single-reader rule: only DVE reads this psum tile
                        fvT = rpool.tile([128, F], DT, tag="fvT")
                        nc.vector.tensor_copy(fvT[:], ps_fwt[:, F:2 * F])
                        frT = rpool.tile([128, F], DT, tag="frT")
                        nc.vector.tensor_copy(frT[:], ps_fwt[:, 0:F])
                        fhT = rpool.tile([128, F], DT, tag="fhT")
                        nc.vector.tensor_copy(fhT[:], ps_fwt[:, 2 * F:3 * F])
                        prod = rpool.tile([128, 3 * F], DT, tag="prod", bufs=3)
                        for d in range(3):
                            nc.vector.tensor_mul(
                                prod[:, d * F:(d + 1) * F],
                                fvT[:],
                                vsb[:, ib * 3 * F + d * F: ib * 3 * F + (d + 1) * F],
                            )
                        ua = unitA[:, ib * JB * 3 + 3 * j: ib * JB * 3 + 3 * j + 3]
                        nc.tensor.matmul(
                            dv_ps[:, 3 * j:3 * j + 3], frT[:], ua,
                            start=False, stop=False, skip_group_check=True,
                        )
                        acol = adjs[:, ib * JB + j: ib * JB + j + 1]
                        for d in range(3):
                            nc.tensor.matmul(
                                dv_ps[:, 3 * j + d:3 * j + d + 1],
                                prod[:, d * F:(d + 1) * F], acol,
                                start=False,
                                stop=(j == JB - 1 and ib == NIB - 1 and d == 2),
                                skip_group_check=True,
                            )
                        nc.tensor.matmul(
                            dh_pre[:, j:j + 1], fhT[:], acol,
                            start=False, stop=(j == JB - 1 and ib == NIB - 1),
                            skip_group_check=True,
                        )

                # ---- dv final: add the b6v*vadj bias, ship out --------
                nc.vector.tensor_add(dv_sb[:], dv_ps[:], bv_vadj[:])

                # ---- dh tail: mul by h, transpose, layernorm ----------
                dhm = wpool.tile([F, JB], DT, tag="dhm")
                nc.vector.tensor_mul(dhm[:], dh_pre[:], hTj[:])
                ps_dt = mps1.tile([JB, F], DT, tag="gm", bufs=1)
                nc.tensor.transpose(ps_dt[:], dhm[:], ident[:])
                sum_col = wpool.tile([JB, 1], DT, tag="sum_col")
                nc.vector.tensor_reduce(sum_col[:], ps_dt[:], axis=AX.X, op=ALU.add)
                mean_col = wpool.tile([JB, 1], DT, tag="mean_col")
                nc.vector.tensor_scalar_mul(mean_col[:], sum_col[:], 1.0 / F)
                cent = wpool.tile([JB, F], DT, tag="cent")
                nc.vector.tensor_scalar_sub(cent[:], ps_dt[:], mean_col[:])
                sq2 = wpool.tile([JB, F], DT, tag="sq2")
                ssq = wpool.tile([JB, 1], DT, tag="ssq")
                nc.scalar.activation(sq2[:], cent[:], AF.Square, accum_out=ssq[:])
                std = wpool.tile([JB, 1], DT, tag="std")
                nc.scalar.activation(std[:], ssq[:], AF.Sqrt, bias=eps_col[:], scale=1.0 / F)
                rstd = wpool.tile([JB, 1], DT, tag="rstd")
                nc.vector.reciprocal(rstd[:], std[:])
                dh1 = wpool.tile([JB, F], DT, tag="dh1")
                nc.vector.scalar_tensor_tensor(
                    dh1[:], cent[:], rstd[:], gamma_bc[:], op0=ALU.mult, op1=ALU.mult
                )
                dh_fin = wpool.tile([JB, F], DT, tag="dh_fin")
                nc.vector.tensor_add(dh_fin[:], dh1[:], beta_bc[:])
                nc.gpsimd.dma_start(out=dh_out_d[:], in_=dh_fin[:])
                nc.gpsimd.dma_start(out=dv_out_d[:], in_=dv_sb[:])

    _split_multiwaits(nc)
    return nc


def _split_multiwaits(nc: bass.Bass):
    """Walrus in this toolchain encodes at most ONE sync-wait per hardware
    instruction ("Too many sync wait commands"). Tile emits multi-wait
    instructions freely, so post-process the BIR: move all but one wait of
    each instruction onto same-engine NOPs inserted immediately before it
    (waits then fire earlier in the same engine stream - strictly safe)."""
    f = nc.m.functions[0]
    for blk in f.blocks:
        il = blk.instructions  # live list backing the block
        idx = 0
        while idx < len(il):
            inst = il[idx]
            si = inst.sync_info
            if si is not None and si.on_wait and len(si.on_wait) > 1:
                waits = list(si.on_wait)
                extra, keep = waits[:-1], waits[-1:]
                nops = []
                for w in extra:
                    bi = nc.engines[inst.engine].nop()
                    nop = bi.ins
                    # nop() appended itself to the current bb; pull it out
                    cur_list = nc.cur_bb.bb.instructions
                    assert cur_list and cur_list[-1].name == nop.name
                    cur_list.pop()
                    nop.sync_info = mybir.SyncInfo(on_wait=[w], on_update=[])
                    nop.bass_nofuse = True
                    nops.append(nop)
                inst.sync_info = mybir.SyncInfo(on_wait=keep, on_update=list(si.on_update))
                for k, nop in enumerate(nops):
                    il.insert(idx + k, nop)
                idx += len(nops)
            idx += 1


_CACHE: dict = {}


def _get_program():
    if "nc" not in _CACHE:
        _CACHE["nc"] = build_program()
    return _CACHE["nc"]


def kernel(h, v, adj, xyz, W1, b1, W2, b2, W3, b3, W4, b4, W5, b5, W6, b6,
           gamma, beta, offset):
    h = np.asarray(h, np.float32)
    v = np.asarray(v, np.float32)
    adj = np.asarray(adj, np.float32)
    xyz = np.asarray(xyz, np.float32)
    f32 = lambda x: np.ascontiguousarray(np.asarray(x, np.float32))

    # W6 column permutation: output channel (f, c) lives at column f*3+c.
    W6 = np.asarray(W6, np.float32)
    b6 = np.asarray(b6, np.float32)
    perm = np.concatenate([np.arange(c, 3 * F, 3) for c in range(3)])
    W6p = np.ascontiguousarray(W6[:, perm])
    b6r, b6v, b6h = b6[0::3], b6[1::3], b6[2::3]

    shared = {
        "W1": f32(W1), "W2": f32(W2), "W3": f32(W3), "W4": f32(W4), "W5": f32(W5),
        "W6p": W6p,
        "b1": f32(b1).reshape(F, 1), "b2": f32(b2).reshape(F, 1),
        "b3": f32(b3).reshape(F, 1), "b4": f32(b4).reshape(F, 1),
        "b5": f32(b5).reshape(F, 1),
        "b6r": f32(b6r).reshape(1, F), "b6v": f32(b6v).reshape(F, 1),
        "b6h": f32(b6h).reshape(1, F),
        "gamma": f32(gamma).reshape(1, F), "beta": f32(beta).reshape(1, F),
        "offr": f32(offset).reshape(1, F),
        "ident": np.ascontiguousarray(np.eye(128, dtype=np.float32)),
    }

    in_maps = []
    for c in range(NC):
        b, q = divmod(c, NC // B)
        js = q * JB
        m = dict(shared)
        m["hT"] = np.ascontiguousarray(h[b].T)
        m["hTj"] = np.ascontiguousarray(h[b, js:js + JB, :].T)
        m["xyzT"] = np.ascontiguousarray(xyz[b].T)
        m["xyzTj"] = np.ascontiguousarray(xyz[b, js:js + JB, :].T)
        m["xyzjd"] = np.ascontiguousarray(xyz[b, js:js + JB, :].reshape(1, JB * 3))
        m["xyzr"] = np.ascontiguousarray(xyz[b])
        m["adjp"] = np.ascontiguousarray(adj[b][:, js:js + JB].reshape(NIB, 128, JB))
        m["vp"] = np.ascontiguousarray(
            v[b].transpose(0, 2, 1).reshape(NIB, 128, 3, F)
        )
        in_maps.append(m)

    nc = _get_program()
    _CACHE["in_maps"] = in_maps
    res = run_bass_kernel_spmd(nc, in_maps, core_ids=list(range(NC))).results

    dh = np.empty((B, N, F), np.float32)
    dv = np.empty((B, N, F, 3), np.float32)
    for c in range(NC):
        b, q = divmod(c, NC // B)
        js = q * JB
        dh[b, js:js + JB] = res[c]["dh_out"]
        dv[b, js:js + JB] = res[c]["dv_out"].reshape(F, JB, 3).transpose(1, 0, 2)
    return dh, dv


# revision 35
# speedup vs baseline: 1.2625x; 1.2625x over previous
"""Trainium2 Bass kernel for DenseEquiMessageBlock (B=2, N=384, F=128).

Sharding: receiver-axis (j) sharding. Core c -> batch b=c//4, receiver block
js=(c%4)*96. Every sum-over-senders contraction is local to a core, so no
collectives are needed; per-core outputs are disjoint slices of dh/dv.

Compute dtype: bf16 for all TensorEngine operands (fp32 matmuls run the PE
at half rate with doubled LDWEIGHTS; bf16 also enables fast weight load and
4x DVE modes). PSUM accumulation stays fp32, as do the geometry setup, the
i-contractions' accumulators, and the LayerNorm tail.

Wait discipline: this toolchain's walrus encodes at most ONE sync-wait per
hardware instruction. Tile emits multi-wait instructions freely, so
_split_multiwaits() post-processes the BIR, moving excess waits onto
same-engine NOPs inserted before the offender. Additionally each PSUM tile
is read by exactly one engine and accumulators use single eternal
accumulation groups (re-opened groups on recycled banks emit PE self-waits).
"""

import numpy as np
import ml_dtypes

import concourse.bass as bass
import concourse.mybir as mybir
from concourse.tile import TileContext
from concourse.bass_utils import run_bass_kernel_spmd

AF = mybir.ActivationFunctionType
ALU = mybir.AluOpType
AX = mybir.AxisListType
DT = mybir.dt.float32
BT = mybir.dt.bfloat16
BF_NP = ml_dtypes.bfloat16

B, N, F = 2, 384, 128
NC = 8
JB = N // 4          # receivers per core
NIB = N // 128       # sender blocks of 128
LN_EPS = 1e-5
RECIP_GUARD = 1e-12


def build_program() -> bass.Bass:
    nc = bass.Bass()

    dp = lambda name, shape, dt=DT: nc.declare_dram_parameter(name, list(shape), dt, isOutput=False)
    hT_d = dp("hT", (F, N), BT)
    hTj_d = dp("hTj", (F, JB), BT)
    hTjf_d = dp("hTjf", (F, JB))
    xyzT_d = dp("xyzT", (3, N))
    xyzTj_d = dp("xyzTj", (3, JB))
    xyzjd_d = dp("xyzjd", (1, JB * 3))
    xyzr_d = dp("xyzr", (N, 3))
    adjp_d = dp("adjp", (NIB, 128, JB), BT)
    adjf_d = dp("adjf", (NIB, 128, JB))
    vp_d = dp("vp", (NIB, 128, 3, F), BT)
    W_d = [dp(f"W{k}", (F, F), BT) for k in (1, 2, 3, 4, 5)]
    W6p_d = dp("W6p", (F, 3 * F), BT)
    b_d = [dp(f"b{k}", (F, 1)) for k in (1, 2, 3, 4, 5)]
    b6r_d = dp("b6r", (1, F), BT)
    b6v_d = dp("b6v", (F, 1))
    b6h_d = dp("b6h", (1, F))
    gamma_d = dp("gamma", (1, F))
    beta_d = dp("beta", (1, F))
    offr_d = dp("offr", (1, F))
    ident_d = dp("ident", (128, 128))
    identb_d = dp("identb", (128, 128), BT)

    dh_out_d = nc.declare_dram_parameter("dh_out", [JB, F], DT, isOutput=True)
    dv_out_d = nc.declare_dram_parameter("dv_out", [F, JB * 3], DT, isOutput=True)

    with TileContext(nc) as tc:
        with (
            tc.tile_pool(name="const", bufs=1) as cpool,
            tc.tile_pool(name="work", bufs=2) as wpool,
            tc.tile_pool(name="red", bufs=2) as rpool,
            tc.tile_pool(name="ps_acc", bufs=1, space="PSUM") as ps_acc,
        ):
            # ---------------- constants in SBUF -----------------------
            ld = lambda tag, shape, d_, dt=DT: (
                lambda t: (nc.sync.dma_start(out=t[:], in_=d_[:]), t)[1]
            )(cpool.tile(list(shape), dt, tag=tag, name=tag))
            ones = cpool.tile([128, N], DT, tag="ones")
            nc.vector.memset(ones[:], 1.0)
            onesb = cpool.tile([128, 1], BT, tag="onesb")
            nc.vector.memset(onesb[:], 1.0)
            ident = ld("ident", (128, 128), ident_d)
            identb = ld("identb", (128, 128), identb_d, BT)
            hT = ld("hT", (F, N), hT_d, BT)
            hTj = ld("hTj", (F, JB), hTj_d, BT)
            hTjf = ld("hTjf", (F, JB), hTjf_d)
            xyzT = ld("xyzT", (3, N), xyzT_d)
            xyzTj = ld("xyzTj", (3, JB), xyzTj_d)
            xyzjd = ld("xyzjd", (1, JB * 3), xyzjd_d)
            xyzib = cpool.tile([128, 3 * NIB], DT, tag="xyzib")
            for ib in range(NIB):
                nc.sync.dma_start(
                    out=xyzib[:, ib * 3:(ib + 1) * 3],
                    in_=xyzr_d[ib * 128:(ib + 1) * 128, :],
                )
            adjs = cpool.tile([128, NIB * JB], BT, tag="adjs")  # col = ib*JB + j
            nc.sync.dma_start(
                out=adjs[:].rearrange("p (b j) -> p b j", b=NIB),
                in_=adjp_d[:].transpose([1, 0, 2]),
            )
            adjf = cpool.tile([128, NIB * JB], DT, tag="adjf")
            nc.sync.dma_start(
                out=adjf[:].rearrange("p (b j) -> p b j", b=NIB),
                in_=adjf_d[:].transpose([1, 0, 2]),
            )
            vsb = cpool.tile([128, NIB * 3 * F], BT, tag="vsb")  # ib*384+d*128+f
            nc.sync.dma_start(
                out=vsb[:].rearrange("p (b d f) -> p b d f", b=NIB, d=3),
                in_=vp_d[:].transpose([1, 0, 2, 3]),
            )
            Ws = [ld(f"W{k + 1}", (F, F), W_d[k], BT) for k in range(5)]
            W6p = ld("W6p", (F, 3 * F), W6p_d, BT)
            bs = [ld(f"b{k + 1}", (F, 1), b_d[k]) for k in range(5)]
            b6r = ld("b6r", (1, F), b6r_d, BT)
            b6v = ld("b6v", (F, 1), b6v_d)
            b6h = ld("b6h", (1, F), b6h_d)
            gamma_r = ld("gamma_r", (1, F), gamma_d)
            beta_r = ld("beta_r", (1, F), beta_d)
            offr = ld("offr", (1, F), offr_d)

            xyzTj_m2 = cpool.tile([3, JB], DT, tag="xyzTj_m2")
            nc.vector.tensor_scalar_mul(xyzTj_m2[:], xyzTj[:], -2.0)
            eps_col = cpool.tile([JB, 1], DT, tag="eps_col")
            nc.vector.memset(eps_col[:], LN_EPS)

            d_rows = cpool.tile([JB, N], DT, tag="d_rows")
            negd = cpool.tile([128, NIB * JB], DT, tag="negd")
            racp = cpool.tile([128, NIB * JB], DT, tag="racp")
            unitA = cpool.tile([128, NIB * JB * 3], BT, tag="unitA")
            s_uA = cpool.tile([1, JB * 3], BT, tag="s_uA")
            adjrowsum = cpool.tile([1, JB], DT, tag="adjrowsum")
            bv_vadj = cpool.tile([F, 3 * JB], DT, tag="bv_vadj")
            phi = cpool.tile([F, N], BT, tag="phi")
            phi_r = cpool.tile([F, JB], BT, tag="phi_r")
            gamma_bc = cpool.tile([JB, F], DT, tag="gamma_bc")
            beta_bc = cpool.tile([JB, F], DT, tag="beta_bc")
            off_bc = cpool.tile([128, F], DT, tag="off_bc")
            dv_sb = cpool.tile([F, JB * 3], DT, tag="dv_sb")

            dh_pre = ps_acc.tile([F, JB], DT, tag="dh_pre")
            dv_ps = ps_acc.tile([F, JB * 3], DT, tag="dv_ps")

            # ================= setup phase ============================
            with tc.tile_pool(name="sps", bufs=5, space="PSUM") as sps:
                # Observer gadget: every engine sees each input-DMA queue once
                # so later instructions carry at most one fresh wait.
                obs_f32 = [ident[:, 0:1], hTjf[:, 0:1], xyzT[:, 0:1],
                           xyzTj[:, 0:1], xyzjd[:, 0:1], adjf[:, 0:1]]
                obs_f32 += [xyzib[:, ib * 3:ib * 3 + 1] for ib in range(NIB)]
                obs_f32 += [t[:, 0:1] for t in bs]
                obs_f32 += [b6v[:, 0:1], b6h[:, 0:1],
                            gamma_r[:, 0:1], beta_r[:, 0:1], offr[:, 0:1]]
                obs_b16 = [identb[:, 0:1], hT[:, 0:1], hTj[:, 0:1], adjs[:, 0:1],
                           vsb[:, 0:1], b6r[:, 0:1], W6p[:, 0:1]]
                obs_b16 += [t[:, 0:1] for t in Ws]
                nobs = len(obs_f32) + len(obs_b16)
                obs_ps = sps.tile([1, nobs + 1], DT, tag="obs", bufs=1)
                obs_dve = wpool.tile([128, nobs], DT, tag="obs_dve", bufs=1)
                obs_act = wpool.tile([128, nobs], DT, tag="obs_act", bufs=1)
                nc.tensor.matmul(obs_ps[:, 0:1], ones[:1, 0:1], ones[:1, 0:1],
                                 start=True, stop=True)
                nc.tensor.matmul(obs_ps[:, 0:1], onesb[:1, 0:1], onesb[:1, 0:1],
                                 start=True, stop=True)
                for c, sl in enumerate(obs_f32 + obs_b16):
                    kp = sl.shape[0]
                    lhs = ones[:kp, 0:1] if c < len(obs_f32) else onesb[:kp, 0:1]
                    nc.tensor.matmul(obs_ps[:, c + 1:c + 2], lhs, sl,
                                     start=True, stop=True)
                    nc.vector.tensor_copy(obs_dve[:kp, c:c + 1], sl)
                    nc.scalar.activation(obs_act[:kp, c:c + 1], sl, AF.Copy)

                # normsq rows |xyz|^2 (senders and local receivers)
                sq = wpool.tile([3, N], DT, tag="sq3")
                nc.scalar.activation(sq[:], xyzT[:], AF.Square)
                ps_nrm = sps.tile([1, N], DT, tag="sps")
                nc.tensor.matmul(ps_nrm[:], ones[:3, 0:1], sq[:], start=True, stop=True)
                normsq = cpool.tile([1, N], DT, tag="normsq")
                nc.scalar.activation(normsq[:], ps_nrm[:], AF.Copy)
                sqj = wpool.tile([3, JB], DT, tag="sqj")
                nc.scalar.activation(sqj[:], xyzTj[:], AF.Square)
                ps_nrj = sps.tile([1, JB], DT, tag="sps")
                nc.tensor.matmul(ps_nrj[:], ones[:3, 0:1], sqj[:], start=True, stop=True)
                normsqj = cpool.tile([1, JB], DT, tag="normsqj")
                nc.scalar.activation(normsqj[:], ps_nrj[:], AF.Copy)

                # d_rows[j, i] = |xyz_{js+j} - xyz_i|
                ps_d2 = sps.tile([JB, N], DT, tag="sps")
                nc.tensor.matmul(ps_d2[:], xyzTj_m2[:], xyzT[:], start=True, stop=False)
                nc.tensor.matmul(ps_d2[:], normsqj[:], ones[:1, :], start=False, stop=False)
                nc.tensor.matmul(ps_d2[:], ones[:1, :JB], normsq[:], start=False, stop=True)
                d2r = wpool.tile([JB, N], DT, tag="d2r")
                nc.vector.tensor_scalar_max(d2r[:], ps_d2[:], 0.0)
                nc.scalar.activation(d_rows[:], d2r[:], AF.Sqrt)

                # d columns (negated) + racp[i,(ib,j)] = adj/(d + guard)
                for ib in range(NIB):
                    ps_t = sps.tile([128, JB], DT, tag="sps")
                    nc.tensor.transpose(
                        ps_t[:], d_rows[:, ib * 128:(ib + 1) * 128], ident[:JB, :JB]
                    )
                    nc.vector.tensor_scalar_mul(
                        negd[:, ib * JB:(ib + 1) * JB], ps_t[:], -1.0
                    )
                    dcol = wpool.tile([128, JB], DT, tag="dcol")
                    nc.vector.tensor_scalar_add(dcol[:], ps_t[:], RECIP_GUARD)
                    rec = wpool.tile([128, JB], DT, tag="rec")
                    nc.vector.reciprocal(rec[:], dcol[:])
                    nc.vector.tensor_mul(
                        racp[:, ib * JB:(ib + 1) * JB], rec[:],
                        adjf[:, ib * JB:(ib + 1) * JB],
                    )

                # unitA[i, (j,d)] per ib = (xyz_j - xyz_i) * racp   (bf16 out)
                for ib in range(NIB):
                    ps_u = sps.tile([128, JB * 3], DT, tag="sps")
                    nc.tensor.matmul(ps_u[:], ones[:1, :128], xyzjd[:], start=True, stop=True)
                    blk = unitA[:, ib * JB * 3:(ib + 1) * JB * 3].rearrange(
                        "p (j d) -> p j d", d=3
                    )
                    xyzrep = (
                        xyzib[:, ib * 3:(ib + 1) * 3].unsqueeze(1).to_broadcast((128, JB, 3))
                    )
                    tmp_u = wpool.tile([128, JB * 3], DT, tag="tmp_u")
                    tmp_v = tmp_u[:].rearrange("p (j d) -> p j d", d=3)
                    nc.vector.tensor_sub(
                        tmp_v, ps_u[:].rearrange("p (j d) -> p j d", d=3), xyzrep
                    )
                    racprep = (
                        racp[:, ib * JB:(ib + 1) * JB].unsqueeze(2).to_broadcast((128, JB, 3))
                    )
                    nc.vector.tensor_mul(blk, tmp_v, racprep)

                # s_uA[1, (j,d)] = sum_i unitA   (bf16 for the rank-1 matmul)
                ps_sa = sps.tile([1, JB * 3], DT, tag="sps")
                for ib in range(NIB):
                    nc.tensor.matmul(
                        ps_sa[:], onesb[:, 0:1], unitA[:, ib * JB * 3:(ib + 1) * JB * 3],
                        start=(ib == 0), stop=(ib == NIB - 1),
                    )
                nc.scalar.activation(s_uA[:], ps_sa[:], AF.Copy)

                # adjrowsum[1, j] = sum_i adj[i, j]
                ps_ar = sps.tile([1, JB], DT, tag="sps")
                for ib in range(NIB):
                    nc.tensor.matmul(
                        ps_ar[:], onesb[:, 0:1], adjs[:, ib * JB:(ib + 1) * JB],
                        start=(ib == 0), stop=(ib == NIB - 1),
                    )
                nc.scalar.activation(adjrowsum[:], ps_ar[:], AF.Copy)

                # bv_vadj[f, (j,d)] = b6v_f * sum_i adj[i,j] v[i,f,d]
                for d in range(3):
                    ps_v = sps.tile([F, JB], DT, tag="sps")
                    for ib in range(NIB):
                        voff = ib * 3 * F + d * F
                        nc.tensor.matmul(
                            ps_v[:], vsb[:, voff:voff + F],
                            adjs[:, ib * JB:(ib + 1) * JB],
                            start=(ib == 0), stop=(ib == NIB - 1),
                        )
                    out_view = bv_vadj[:].rearrange("p (j d) -> p j d", d=3)[:, :, d]
                    nc.vector.tensor_scalar_mul(out_view, ps_v[:], b6v[:])

                # phi over all senders; phi_r over local receivers (bf16)
                ps_p1 = sps.tile([F, N], DT, tag="sps")
                nc.tensor.matmul(ps_p1[:], Ws[0][:], hT[:], start=True, stop=True)
                u1 = wpool.tile([F, N], BT, tag="u1")
                nc.scalar.activation(u1[:], ps_p1[:], AF.Silu, bias=bs[0][:])
                ps_p2 = sps.tile([F, N], DT, tag="sps")
                nc.tensor.matmul(ps_p2[:], Ws[1][:], u1[:], start=True, stop=True)
                nc.vector.tensor_scalar_add(phi[:], ps_p2[:], bs[1][:])

                ps_q1 = sps.tile([F, JB], DT, tag="sps")
                nc.tensor.matmul(ps_q1[:], Ws[0][:], hTj[:], start=True, stop=True)
                u1r = wpool.tile([F, JB], BT, tag="u1r")
                nc.scalar.activation(u1r[:], ps_q1[:], AF.Silu, bias=bs[0][:])
                ps_q2 = sps.tile([F, JB], DT, tag="sps")
                nc.tensor.matmul(ps_q2[:], Ws[1][:], u1r[:], start=True, stop=True)
                nc.vector.tensor_scalar_add(phi_r[:], ps_q2[:], bs[1][:])

                # broadcast rows -> [JB, F] / [128, F]
                ps_g = sps.tile([JB, F], DT, tag="sps")
                nc.tensor.matmul(ps_g[:], ones[:1, :JB], gamma_r[:], start=True, stop=True)
                nc.scalar.activation(gamma_bc[:], ps_g[:], AF.Copy)
                ps_b = sps.tile([JB, F], DT, tag="sps")
                nc.tensor.matmul(ps_b[:], ones[:1, :JB], beta_r[:], start=True, stop=True)
                nc.scalar.activation(beta_bc[:], ps_b[:], AF.Copy)
                ps_o = sps.tile([128, F], DT, tag="sps")
                nc.tensor.matmul(ps_o[:], ones[:1, :128], offr[:], start=True, stop=True)
                nc.scalar.activation(off_bc[:], ps_o[:], AF.Copy)

                # accumulator inits (single eternal groups)
                nc.tensor.matmul(dh_pre[:], b6h[:], adjrowsum[:], start=True, stop=False,
                                 skip_group_check=True)
                nc.tensor.matmul(dv_ps[:], b6r[:], s_uA[:], start=True, stop=False,
                                 skip_group_check=True)
                # let ACT observe its own off_bc drain + DVE's negd before j=0
                nc.scalar.activation(obs_act[:, 0:1], off_bc[:, 0:1], AF.Copy)
                nc.scalar.activation(obs_act[:, 1:2], negd[:, 0:1], AF.Copy)

            # ================= main loop over receivers ===============
            with (
                tc.tile_pool(name="mps1", bufs=1, space="PSUM") as mps1,
                tc.tile_pool(name="mps2", bufs=2, space="PSUM") as mps2,
            ):
                for j in range(JB):
                    # rbf[f, i] = exp(-(d_j[i]-off_f)^2): (off-d)^2 transposed
                    # [i, f] with d as per-partition ACT bias, PE-transpose back
                    ps_tq = mps1.tile([F, N], BT, tag="tsq", bufs=1)
                    for ib in range(NIB):
                        tsqT = rpool.tile([128, F], BT, tag="tsqT", bufs=3)
                        nc.scalar.activation(
                            tsqT[:], off_bc[:], AF.Square,
                            bias=negd[:, ib * JB + j: ib * JB + j + 1],
                        )
                        nc.tensor.transpose(
                            ps_tq[:, ib * 128:(ib + 1) * 128], tsqT[:], identb[:]
                        )
                    rbf = wpool.tile([F, N], BT, tag="rbf")
                    nc.scalar.activation(rbf[:], ps_tq[:], AF.Exp, scale=-1.0)

                    # w = MLP2(rbf)
                    ps_u3 = mps1.tile([F, N], DT, tag="u3", bufs=1)
                    nc.tensor.matmul(ps_u3[:], Ws[2][:], rbf[:], start=True, stop=True)
                    u3 = wpool.tile([F, N], BT, tag="u3")
                    nc.scalar.activation(u3[:], ps_u3[:], AF.Silu, bias=bs[2][:])
                    ps_w = mps1.tile([F, N], DT, tag="w", bufs=1)
                    nc.tensor.matmul(ps_w[:], Ws[3][:], u3[:], start=True, stop=True)
                    wt = wpool.tile([F, N], BT, tag="wt")
                    nc.scalar.activation(wt[:], ps_w[:], AF.Identity, bias=bs[3][:])

                    # m = w * phi_j * phi_i ; g = silu(W5.T m + b5)
                    m = wpool.tile([F, N], BT, tag="m")
                    nc.vector.scalar_tensor_tensor(
                        m[:], wt[:], phi_r[:, j:j + 1], phi[:],
                        op0=ALU.mult, op1=ALU.mult,
                    )
                    ps_gm = mps1.tile([F, N], DT, tag="gm", bufs=1)
                    nc.tensor.matmul(ps_gm[:], Ws[4][:], m[:], start=True, stop=True)
                    g = wpool.tile([F, N], BT, tag="g")
                    nc.scalar.activation(g[:], ps_gm[:], AF.Silu, bias=bs[4][:])

                    for ib in range(NIB):
                        ps_fwt = mps2.tile([128, 3 * F], DT, tag="fwt")
                        nc.tensor.matmul(
                            ps_fwt[:], g[:, ib * 128:(ib + 1) * 128], W6p[:],
                            start=True, stop=True,
                        )
                        # single merged psum->sbuf extraction (DVE only reader)
                        fw = rpool.tile([128, 3 * F], BT, tag="fw", bufs=3)
                        nc.vector.tensor_copy(fw[:], ps_fwt[:])
                        prod = rpool.tile([128, 3 * F], BT, tag="prod", bufs=3)
                        for d in range(3):
                            nc.vector.tensor_mul(
                                prod[:, d * F:(d + 1) * F],
                                fw[:, F:2 * F],
                                vsb[:, ib * 3 * F + d * F: ib * 3 * F + (d + 1) * F],
                            )
                        ua = unitA[:, ib * JB * 3 + 3 * j: ib * JB * 3 + 3 * j + 3]
                        nc.tensor.matmul(
                            dv_ps[:, 3 * j:3 * j + 3], fw[:, 0:F], ua,
                            start=False, stop=False, skip_group_check=True,
                        )
                        acol = adjs[:, ib * JB + j: ib * JB + j + 1]
                        for d in range(3):
                            nc.tensor.matmul(
                                dv_ps[:, 3 * j + d:3 * j + d + 1],
                                prod[:, d * F:(d + 1) * F], acol,
                                start=False,
                                stop=(j == JB - 1 and ib == NIB - 1 and d == 2),
                                skip_group_check=True,
                            )
                        nc.tensor.matmul(
                            dh_pre[:, j:j + 1], fw[:, 2 * F:3 * F], acol,
                            start=False, stop=(j == JB - 1 and ib == NIB - 1),
                            skip_group_check=True,
                        )

                # ---- dv final: add the b6v*vadj bias, ship out --------
                nc.vector.tensor_add(dv_sb[:], dv_ps[:], bv_vadj[:])

                # ---- dh tail: mul by h, transpose, layernorm ----------
                dhm = wpool.tile([F, JB], DT, tag="dhm")
                nc.vector.tensor_mul(dhm[:], dh_pre[:], hTjf[:])
                ps_dt = mps1.tile([JB, F], DT, tag="gm", bufs=1)
                nc.tensor.transpose(ps_dt[:], dhm[:], ident[:])
                sum_col = wpool.tile([JB, 1], DT, tag="sum_col")
                nc.vector.tensor_reduce(sum_col[:], ps_dt[:], axis=AX.X, op=ALU.add)
                mean_col = wpool.tile([JB, 1], DT, tag="mean_col")
                nc.vector.tensor_scalar_mul(mean_col[:], sum_col[:], 1.0 / F)
                cent = wpool.tile([JB, F], DT, tag="cent")
                nc.vector.tensor_scalar_sub(cent[:], ps_dt[:], mean_col[:])
                sq2 = wpool.tile([JB, F], DT, tag="sq2")
                ssq = wpool.tile([JB, 1], DT, tag="ssq")
                nc.scalar.activation(sq2[:], cent[:], AF.Square, accum_out=ssq[:])
                std = wpool.tile([JB, 1], DT, tag="std")
                nc.scalar.activation(std[:], ssq[:], AF.Sqrt, bias=eps_col[:], scale=1.0 / F)
                rstd = wpool.tile([JB, 1], DT, tag="rstd")
                nc.vector.reciprocal(rstd[:], std[:])
                dh1 = wpool.tile([JB, F], DT, tag="dh1")
                nc.vector.scalar_tensor_tensor(
                    dh1[:], cent[:], rstd[:], gamma_bc[:], op0=ALU.mult, op1=ALU.mult
                )
                dh_fin = wpool.tile([JB, F], DT, tag="dh_fin")
                nc.vector.tensor_add(dh_fin[:], dh1[:], beta_bc[:])
                nc.gpsimd.dma_start(out=dh_out_d[:], in_=dh_fin[:])
                nc.gpsimd.dma_start(out=dv_out_d[:], in_=dv_sb[:])

    _split_multiwaits(nc)
    return nc


def _split_multiwaits(nc: bass.Bass):
    """Move all but one sync-wait of each instruction onto same-engine NOPs
    inserted immediately before it (walrus allows one wait per hw inst)."""
    f = nc.m.functions[0]
    for blk in f.blocks:
        il = blk.instructions  # live list backing the block
        idx = 0
        while idx < len(il):
            inst = il[idx]
            si = inst.sync_info
            if si is not None and si.on_wait and len(si.on_wait) > 1:
                waits = list(si.on_wait)
                extra, keep = waits[:-1], waits[-1:]
                nops = []
                for w in extra:
                    bi = nc.engines[inst.engine].nop()
                    nop = bi.ins
                    cur_list = nc.cur_bb.bb.instructions
                    assert cur_list and cur_list[-1].name == nop.name
                    cur_list.pop()
                    nop.sync_info = mybir.SyncInfo(on_wait=[w], on_update=[])
                    nop.bass_nofuse = True
                    nops.append(nop)
                inst.sync_info = mybir.SyncInfo(on_wait=keep, on_update=list(si.on_update))
                for k, nop in enumerate(nops):
                    il.insert(idx + k, nop)
                idx += len(nops)
            idx += 1


_CACHE: dict = {}


def _get_program():
    if "nc" not in _CACHE:
        _CACHE["nc"] = build_program()
    return _CACHE["nc"]


def kernel(h, v, adj, xyz, W1, b1, W2, b2, W3, b3, W4, b4, W5, b5, W6, b6,
           gamma, beta, offset):
    h = np.asarray(h, np.float32)
    v = np.asarray(v, np.float32)
    adj = np.asarray(adj, np.float32)
    xyz = np.asarray(xyz, np.float32)
    f32 = lambda x: np.ascontiguousarray(np.asarray(x, np.float32))
    b16 = lambda x: np.ascontiguousarray(np.asarray(x, np.float32).astype(BF_NP))

    # W6 column permutation: output channel (f, c) lives at column f*3+c.
    W6 = np.asarray(W6, np.float32)
    b6 = np.asarray(b6, np.float32)
    perm = np.concatenate([np.arange(c, 3 * F, 3) for c in range(3)])
    W6p = W6[:, perm]
    b6r, b6v, b6h = b6[0::3], b6[1::3], b6[2::3]

    shared = {
        "W1": b16(W1), "W2": b16(W2), "W3": b16(W3), "W4": b16(W4), "W5": b16(W5),
        "W6p": b16(W6p),
        "b1": f32(b1).reshape(F, 1), "b2": f32(b2).reshape(F, 1),
        "b3": f32(b3).reshape(F, 1), "b4": f32(b4).reshape(F, 1),
        "b5": f32(b5).reshape(F, 1),
        "b6r": b16(b6r).reshape(F)[None, :], "b6v": f32(b6v).reshape(F, 1),
        "b6h": f32(b6h).reshape(1, F),
        "gamma": f32(gamma).reshape(1, F), "beta": f32(beta).reshape(1, F),
        "offr": f32(offset).reshape(1, F),
        "ident": np.ascontiguousarray(np.eye(128, dtype=np.float32)),
        "identb": np.ascontiguousarray(np.eye(128, dtype=np.float32).astype(BF_NP)),
    }

    in_maps = []
    for c in range(NC):
        b, q = divmod(c, NC // B)
        js = q * JB
        m = dict(shared)
        m["hT"] = b16(h[b].T)
        m["hTj"] = b16(h[b, js:js + JB, :].T)
        m["hTjf"] = f32(h[b, js:js + JB, :].T)
        m["xyzT"] = f32(xyz[b].T)
        m["xyzTj"] = f32(xyz[b, js:js + JB, :].T)
        m["xyzjd"] = f32(xyz[b, js:js + JB, :].reshape(1, JB * 3))
        m["xyzr"] = f32(xyz[b])
        m["adjp"] = b16(adj[b][:, js:js + JB].reshape(NIB, 128, JB))
        m["adjf"] = f32(adj[b][:, js:js + JB].reshape(NIB, 128, JB))
        m["vp"] = b16(v[b].transpose(0, 2, 1).reshape(NIB, 128, 3, F))
        in_maps.append(m)

    nc = _get_program()
    _CACHE["in_maps"] = in_maps
    res = run_bass_kernel_spmd(nc, in_maps, core_ids=list(range(NC))).results

    dh = np.empty((B, N, F), np.float32)
    dv = np.empty((B, N, F, 3), np.float32)
    for c in range(NC):
        b, q = divmod(c, NC // B)
        js = q * JB
        dh[b, js:js + JB] = res[c]["dh_out"]
        dv[b, js:js + JB] = res[c]["dv_out"].reshape(F, JB, 3).transpose(1, 0, 2)
    return dh, dv


# revision 38
# speedup vs baseline: 1.3156x; 1.0421x over previous
"""Trainium2 Bass kernel for DenseEquiMessageBlock (B=2, N=384, F=128).

Sharding: receiver-axis (j) sharding. Core c -> batch b=c//4, receiver block
js=(c%4)*96. Every sum-over-senders contraction is local to a core, so no
collectives are needed; per-core outputs are disjoint slices of dh/dv.

Compute dtype: bf16 for all TensorEngine operands (fp32 matmuls run the PE
at half rate with doubled LDWEIGHTS; bf16 also enables fast weight load and
4x DVE modes). PSUM accumulation stays fp32, as do the geometry setup, the
i-contractions' accumulators, and the LayerNorm tail.

Wait discipline: this toolchain's walrus encodes at most ONE sync-wait per
hardware instruction. Tile emits multi-wait instructions freely, so
_split_multiwaits() post-processes the BIR, moving excess waits onto
same-engine NOPs inserted before the offender. Additionally each PSUM tile
is read by exactly one engine and accumulators use single eternal
accumulation groups (re-opened groups on recycled banks emit PE self-waits).
"""

import numpy as np
import ml_dtypes

import concourse.bass as bass
import concourse.mybir as mybir
from concourse.tile import TileContext
from concourse.bass_utils import run_bass_kernel_spmd

AF = mybir.ActivationFunctionType
ALU = mybir.AluOpType
AX = mybir.AxisListType
DT = mybir.dt.float32
BT = mybir.dt.bfloat16
BF_NP = ml_dtypes.bfloat16

B, N, F = 2, 384, 128
NC = 8
JB = N // 4          # receivers per core
NIB = N // 128       # sender blocks of 128
LN_EPS = 1e-5
RECIP_GUARD = 1e-12


def build_program() -> bass.Bass:
    nc = bass.Bass()

    dp = lambda name, shape, dt=DT: nc.declare_dram_parameter(name, list(shape), dt, isOutput=False)
    hT_d = dp("hT", (F, N), BT)
    hTj_d = dp("hTj", (F, JB), BT)
    hTjf_d = dp("hTjf", (F, JB))
    xyzT_d = dp("xyzT", (3, N))
    xyzTj_d = dp("xyzTj", (3, JB))
    xyzjd_d = dp("xyzjd", (1, JB * 3))
    xyzr_d = dp("xyzr", (N, 3))
    adjp_d = dp("adjp", (NIB, 128, JB), BT)
    adjf_d = dp("adjf", (NIB, 128, JB))
    vp_d = dp("vp", (NIB, 128, 3, F), BT)
    W_d = [dp(f"W{k}", (F, F), BT) for k in (1, 2, 3, 4, 5)]
    W6p_d = dp("W6p", (F, 3 * F), BT)
    b_d = [dp(f"b{k}", (F, 1)) for k in (1, 2, 3, 4, 5)]
    b6r_d = dp("b6r", (1, F), BT)
    b6v_d = dp("b6v", (F, 1))
    b6h_d = dp("b6h", (1, F))
    gamma_d = dp("gamma", (1, F))
    beta_d = dp("beta", (1, F))
    offr_d = dp("offr", (1, F))
    ident_d = dp("ident", (128, 128))
    identb_d = dp("identb", (128, 128), BT)

    dh_out_d = nc.declare_dram_parameter("dh_out", [JB, F], DT, isOutput=True)
    dv_out_d = nc.declare_dram_parameter("dv_out", [F, JB * 3], DT, isOutput=True)

    with TileContext(nc) as tc:
        with (
            tc.tile_pool(name="const", bufs=1) as cpool,
            tc.tile_pool(name="work", bufs=2) as wpool,
            tc.tile_pool(name="red", bufs=2) as rpool,
            tc.tile_pool(name="ps_acc", bufs=1, space="PSUM") as ps_acc,
        ):
            # ---------------- constants in SBUF -----------------------
            ld = lambda tag, shape, d_, dt=DT: (
                lambda t: (nc.sync.dma_start(out=t[:], in_=d_[:]), t)[1]
            )(cpool.tile(list(shape), dt, tag=tag, name=tag))
            ones = cpool.tile([128, N], DT, tag="ones")
            nc.vector.memset(ones[:], 1.0)
            onesb = cpool.tile([128, 1], BT, tag="onesb")
            nc.vector.memset(onesb[:], 1.0)
            ident = ld("ident", (128, 128), ident_d)
            identb = ld("identb", (128, 128), identb_d, BT)
            hT = ld("hT", (F, N), hT_d, BT)
            hTj = ld("hTj", (F, JB), hTj_d, BT)
            hTjf = ld("hTjf", (F, JB), hTjf_d)
            xyzT = ld("xyzT", (3, N), xyzT_d)
            xyzTj = ld("xyzTj", (3, JB), xyzTj_d)
            xyzjd = ld("xyzjd", (1, JB * 3), xyzjd_d)
            xyzib = cpool.tile([128, 3 * NIB], DT, tag="xyzib")
            for ib in range(NIB):
                nc.sync.dma_start(
                    out=xyzib[:, ib * 3:(ib + 1) * 3],
                    in_=xyzr_d[ib * 128:(ib + 1) * 128, :],
                )
            adjs = cpool.tile([128, NIB * JB], BT, tag="adjs")  # col = ib*JB + j
            nc.sync.dma_start(
                out=adjs[:].rearrange("p (b j) -> p b j", b=NIB),
                in_=adjp_d[:].transpose([1, 0, 2]),
            )
            adjf = cpool.tile([128, NIB * JB], DT, tag="adjf")
            nc.sync.dma_start(
                out=adjf[:].rearrange("p (b j) -> p b j", b=NIB),
                in_=adjf_d[:].transpose([1, 0, 2]),
            )
            vsb = cpool.tile([128, NIB * 3 * F], BT, tag="vsb")  # ib*384+d*128+f
            nc.sync.dma_start(
                out=vsb[:].rearrange("p (b d f) -> p b d f", b=NIB, d=3),
                in_=vp_d[:].transpose([1, 0, 2, 3]),
            )
            Ws = [ld(f"W{k + 1}", (F, F), W_d[k], BT) for k in range(5)]
            W6p = ld("W6p", (F, 3 * F), W6p_d, BT)
            bs = [ld(f"b{k + 1}", (F, 1), b_d[k]) for k in range(5)]
            b6r = ld("b6r", (1, F), b6r_d, BT)
            b6v = ld("b6v", (F, 1), b6v_d)
            b6h = ld("b6h", (1, F), b6h_d)
            gamma_r = ld("gamma_r", (1, F), gamma_d)
            beta_r = ld("beta_r", (1, F), beta_d)
            offr = ld("offr", (1, F), offr_d)

            xyzTj_m2 = cpool.tile([3, JB], DT, tag="xyzTj_m2")
            nc.vector.tensor_scalar_mul(xyzTj_m2[:], xyzTj[:], -2.0)
            eps_col = cpool.tile([JB, 1], DT, tag="eps_col")
            nc.vector.memset(eps_col[:], LN_EPS)

            d_rows = cpool.tile([JB, N], DT, tag="d_rows")
            negd = cpool.tile([128, NIB * JB], DT, tag="negd")
            racp = cpool.tile([128, NIB * JB], DT, tag="racp")
            unitA = cpool.tile([128, NIB * JB * 3], BT, tag="unitA")
            s_uA = cpool.tile([1, JB * 3], BT, tag="s_uA")
            adjrowsum = cpool.tile([1, JB], DT, tag="adjrowsum")
            bv_vadj = cpool.tile([F, 3 * JB], DT, tag="bv_vadj")
            phi = cpool.tile([F, N], BT, tag="phi")
            phi_r = cpool.tile([F, JB], BT, tag="phi_r")
            gamma_bc = cpool.tile([JB, F], DT, tag="gamma_bc")
            beta_bc = cpool.tile([JB, F], DT, tag="beta_bc")
            off_bc = cpool.tile([128, F], DT, tag="off_bc")
            dv_sb = cpool.tile([F, JB * 3], DT, tag="dv_sb")

            dh_pre = ps_acc.tile([F, JB], DT, tag="dh_pre")
            dv_ps = ps_acc.tile([F, JB * 3], DT, tag="dv_ps")

            # ================= setup phase ============================
            with tc.tile_pool(name="sps", bufs=5, space="PSUM") as sps:
                # Observer gadget: every engine sees each input-DMA queue once
                # so later instructions carry at most one fresh wait.
                obs_f32 = [ident[:, 0:1], hTjf[:, 0:1], xyzT[:, 0:1],
                           xyzTj[:, 0:1], xyzjd[:, 0:1], adjf[:, 0:1]]
                obs_f32 += [xyzib[:, ib * 3:ib * 3 + 1] for ib in range(NIB)]
                obs_f32 += [t[:, 0:1] for t in bs]
                obs_f32 += [b6v[:, 0:1], b6h[:, 0:1],
                            gamma_r[:, 0:1], beta_r[:, 0:1], offr[:, 0:1]]
                obs_b16 = [identb[:, 0:1], hT[:, 0:1], hTj[:, 0:1], adjs[:, 0:1],
                           vsb[:, 0:1], b6r[:, 0:1], W6p[:, 0:1]]
                obs_b16 += [t[:, 0:1] for t in Ws]
                nobs = len(obs_f32) + len(obs_b16)
                obs_ps = sps.tile([1, nobs + 1], DT, tag="obs", bufs=1)
                obs_dve = wpool.tile([128, nobs], DT, tag="obs_dve", bufs=1)
                obs_act = wpool.tile([128, nobs], DT, tag="obs_act", bufs=1)
                nc.tensor.matmul(obs_ps[:, 0:1], ones[:1, 0:1], ones[:1, 0:1],
                                 start=True, stop=True)
                nc.tensor.matmul(obs_ps[:, 0:1], onesb[:1, 0:1], onesb[:1, 0:1],
                                 start=True, stop=True)
                for c, sl in enumerate(obs_f32 + obs_b16):
                    kp = sl.shape[0]
                    lhs = ones[:kp, 0:1] if c < len(obs_f32) else onesb[:kp, 0:1]
                    nc.tensor.matmul(obs_ps[:, c + 1:c + 2], lhs, sl,
                                     start=True, stop=True)
                    nc.vector.tensor_copy(obs_dve[:kp, c:c + 1], sl)
                    nc.scalar.activation(obs_act[:kp, c:c + 1], sl, AF.Copy)

                # normsq rows |xyz|^2 (senders and local receivers)
                sq = wpool.tile([3, N], DT, tag="sq3")
                nc.scalar.activation(sq[:], xyzT[:], AF.Square)
                ps_nrm = sps.tile([1, N], DT, tag="sps")
                nc.tensor.matmul(ps_nrm[:], ones[:3, 0:1], sq[:], start=True, stop=True)
                normsq = cpool.tile([1, N], DT, tag="normsq")
                nc.scalar.activation(normsq[:], ps_nrm[:], AF.Copy)
                sqj = wpool.tile([3, JB], DT, tag="sqj")
                nc.scalar.activation(sqj[:], xyzTj[:], AF.Square)
                ps_nrj = sps.tile([1, JB], DT, tag="sps")
                nc.tensor.matmul(ps_nrj[:], ones[:3, 0:1], sqj[:], start=True, stop=True)
                normsqj = cpool.tile([1, JB], DT, tag="normsqj")
                nc.scalar.activation(normsqj[:], ps_nrj[:], AF.Copy)

                # d_rows[j, i] = |xyz_{js+j} - xyz_i|
                ps_d2 = sps.tile([JB, N], DT, tag="sps")
                nc.tensor.matmul(ps_d2[:], xyzTj_m2[:], xyzT[:], start=True, stop=False)
                nc.tensor.matmul(ps_d2[:], normsqj[:], ones[:1, :], start=False, stop=False)
                nc.tensor.matmul(ps_d2[:], ones[:1, :JB], normsq[:], start=False, stop=True)
                d2r = wpool.tile([JB, N], DT, tag="d2r")
                nc.vector.tensor_scalar_max(d2r[:], ps_d2[:], 0.0)
                nc.scalar.activation(d_rows[:], d2r[:], AF.Sqrt)

                # d columns (negated) + racp[i,(ib,j)] = adj/(d + guard)
                for ib in range(NIB):
                    ps_t = sps.tile([128, JB], DT, tag="sps")
                    nc.tensor.transpose(
                        ps_t[:], d_rows[:, ib * 128:(ib + 1) * 128], ident[:JB, :JB]
                    )
                    nc.vector.tensor_scalar_mul(
                        negd[:, ib * JB:(ib + 1) * JB], ps_t[:], -1.0
                    )
                    dcol = wpool.tile([128, JB], DT, tag="dcol")
                    nc.vector.tensor_scalar_add(dcol[:], ps_t[:], RECIP_GUARD)
                    rec = wpool.tile([128, JB], DT, tag="rec")
                    nc.vector.reciprocal(rec[:], dcol[:])
                    nc.vector.tensor_mul(
                        racp[:, ib * JB:(ib + 1) * JB], rec[:],
                        adjf[:, ib * JB:(ib + 1) * JB],
                    )

                # unitA[i, (j,d)] per ib = (xyz_j - xyz_i) * racp   (bf16 out)
                for ib in range(NIB):
                    ps_u = sps.tile([128, JB * 3], DT, tag="sps")
                    nc.tensor.matmul(ps_u[:], ones[:1, :128], xyzjd[:], start=True, stop=True)
                    blk = unitA[:, ib * JB * 3:(ib + 1) * JB * 3].rearrange(
                        "p (j d) -> p j d", d=3
                    )
                    xyzrep = (
                        xyzib[:, ib * 3:(ib + 1) * 3].unsqueeze(1).to_broadcast((128, JB, 3))
                    )
                    tmp_u = wpool.tile([128, JB * 3], DT, tag="tmp_u")
                    tmp_v = tmp_u[:].rearrange("p (j d) -> p j d", d=3)
                    nc.vector.tensor_sub(
                        tmp_v, ps_u[:].rearrange("p (j d) -> p j d", d=3), xyzrep
                    )
                    racprep = (
                        racp[:, ib * JB:(ib + 1) * JB].unsqueeze(2).to_broadcast((128, JB, 3))
                    )
                    nc.vector.tensor_mul(blk, tmp_v, racprep)

                # s_uA[1, (j,d)] = sum_i unitA   (bf16 for the rank-1 matmul)
                ps_sa = sps.tile([1, JB * 3], DT, tag="sps")
                for ib in range(NIB):
                    nc.tensor.matmul(
                        ps_sa[:], onesb[:, 0:1], unitA[:, ib * JB * 3:(ib + 1) * JB * 3],
                        start=(ib == 0), stop=(ib == NIB - 1),
                    )
                nc.scalar.activation(s_uA[:], ps_sa[:], AF.Copy)

                # adjrowsum[1, j] = sum_i adj[i, j]
                ps_ar = sps.tile([1, JB], DT, tag="sps")
                for ib in range(NIB):
                    nc.tensor.matmul(
                        ps_ar[:], onesb[:, 0:1], adjs[:, ib * JB:(ib + 1) * JB],
                        start=(ib == 0), stop=(ib == NIB - 1),
                    )
                nc.scalar.activation(adjrowsum[:], ps_ar[:], AF.Copy)

                # bv_vadj[f, (j,d)] = b6v_f * sum_i adj[i,j] v[i,f,d]
                for d in range(3):
                    ps_v = sps.tile([F, JB], DT, tag="sps")
                    for ib in range(NIB):
                        voff = ib * 3 * F + d * F
                        nc.tensor.matmul(
                            ps_v[:], vsb[:, voff:voff + F],
                            adjs[:, ib * JB:(ib + 1) * JB],
                            start=(ib == 0), stop=(ib == NIB - 1),
                        )
                    out_view = bv_vadj[:].rearrange("p (j d) -> p j d", d=3)[:, :, d]
                    nc.vector.tensor_scalar_mul(out_view, ps_v[:], b6v[:])

                # phi over all senders; phi_r over local receivers (bf16)
                ps_p1 = sps.tile([F, N], DT, tag="sps")
                nc.tensor.matmul(ps_p1[:], Ws[0][:], hT[:], start=True, stop=True)
                u1 = wpool.tile([F, N], BT, tag="u1")
                nc.scalar.activation(u1[:], ps_p1[:], AF.Silu, bias=bs[0][:])
                ps_p2 = sps.tile([F, N], DT, tag="sps")
                nc.tensor.matmul(ps_p2[:], Ws[1][:], u1[:], start=True, stop=True)
                nc.vector.tensor_scalar_add(phi[:], ps_p2[:], bs[1][:])

                ps_q1 = sps.tile([F, JB], DT, tag="sps")
                nc.tensor.matmul(ps_q1[:], Ws[0][:], hTj[:], start=True, stop=True)
                u1r = wpool.tile([F, JB], BT, tag="u1r")
                nc.scalar.activation(u1r[:], ps_q1[:], AF.Silu, bias=bs[0][:])
                ps_q2 = sps.tile([F, JB], DT, tag="sps")
                nc.tensor.matmul(ps_q2[:], Ws[1][:], u1r[:], start=True, stop=True)
                nc.vector.tensor_scalar_add(phi_r[:], ps_q2[:], bs[1][:])

                # broadcast rows -> [JB, F] / [128, F]
                ps_g = sps.tile([JB, F], DT, tag="sps")
                nc.tensor.matmul(ps_g[:], ones[:1, :JB], gamma_r[:], start=True, stop=True)
                nc.scalar.activation(gamma_bc[:], ps_g[:], AF.Copy)
                ps_b = sps.tile([JB, F], DT, tag="sps")
                nc.tensor.matmul(ps_b[:], ones[:1, :JB], beta_r[:], start=True, stop=True)
                nc.scalar.activation(beta_bc[:], ps_b[:], AF.Copy)
                ps_o = sps.tile([128, F], DT, tag="sps")
                nc.tensor.matmul(ps_o[:], ones[:1, :128], offr[:], start=True, stop=True)
                nc.scalar.activation(off_bc[:], ps_o[:], AF.Copy)

                # accumulator inits (single eternal groups)
                nc.tensor.matmul(dh_pre[:], b6h[:], adjrowsum[:], start=True, stop=False,
                                 skip_group_check=True)
                nc.tensor.matmul(dv_ps[:], b6r[:], s_uA[:], start=True, stop=False,
                                 skip_group_check=True)
                # let ACT observe its own off_bc drain + DVE's negd before j=0
                nc.scalar.activation(obs_act[:, 0:1], off_bc[:, 0:1], AF.Copy)
                nc.scalar.activation(obs_act[:, 1:2], negd[:, 0:1], AF.Copy)

            # ================= main loop over receivers ===============
            with (
                tc.tile_pool(name="mps1", bufs=1, space="PSUM") as mps1,
                tc.tile_pool(name="mps2", bufs=2, space="PSUM") as mps2,
            ):
                for jp in range(0, JB, 2):
                  # Receiver pairs: batch the exp-set ACT ops (Square, Exp)
                  # of both receivers before the silu-set ops, halving the
                  # ~1.3us ACT table reloads from 2 per j to 1 per j.
                  rbfs = {}
                  for j in (jp, jp + 1):
                    ps_tq = mps1.tile([F, N], BT, tag="tsq", bufs=2)
                    for ib in range(NIB):
                        tsqT = rpool.tile([128, F], BT, tag="tsqT", bufs=6)
                        nc.scalar.activation(
                            tsqT[:], off_bc[:], AF.Square,
                            bias=negd[:, ib * JB + j: ib * JB + j + 1],
                        )
                        nc.tensor.transpose(
                            ps_tq[:, ib * 128:(ib + 1) * 128], tsqT[:], identb[:]
                        )
                    rbfs[j] = ps_tq
                  for j in (jp, jp + 1):
                    rbf = wpool.tile([F, N], BT, tag="rbf")
                    nc.scalar.activation(rbf[:], rbfs[j][:], AF.Exp, scale=-1.0)
                    rbfs[j] = rbf
                  for j in (jp, jp + 1):
                    rbf = rbfs[j]
                    # w = MLP2(rbf)
                    ps_u3 = mps1.tile([F, N], DT, tag="u3", bufs=1)
                    nc.tensor.matmul(ps_u3[:], Ws[2][:], rbf[:], start=True, stop=True)
                    u3 = wpool.tile([F, N], BT, tag="u3")
                    nc.scalar.activation(u3[:], ps_u3[:], AF.Silu, bias=bs[2][:])
                    ps_w = mps1.tile([F, N], DT, tag="w", bufs=1)
                    nc.tensor.matmul(ps_w[:], Ws[3][:], u3[:], start=True, stop=True)
                    wt = wpool.tile([F, N], BT, tag="wt")
                    nc.vector.tensor_scalar_add(wt[:], ps_w[:], bs[3][:])

                    # m = w * phi_j * phi_i ; g = silu(W5.T m + b5)
                    m = wpool.tile([F, N], BT, tag="m")
                    nc.vector.scalar_tensor_tensor(
                        m[:], wt[:], phi_r[:, j:j + 1], phi[:],
                        op0=ALU.mult, op1=ALU.mult,
                    )
                    ps_gm = mps1.tile([F, N], DT, tag="gm", bufs=1)
                    nc.tensor.matmul(ps_gm[:], Ws[4][:], m[:], start=True, stop=True)
                    g = wpool.tile([F, N], BT, tag="g")
                    nc.scalar.activation(g[:], ps_gm[:], AF.Silu, bias=bs[4][:])

                    for ib in range(NIB):
                        ps_fwt = mps2.tile([128, 3 * F], DT, tag="fwt", bufs=1)
                        nc.tensor.matmul(
                            ps_fwt[:], g[:, ib * 128:(ib + 1) * 128], W6p[:],
                            start=True, stop=True,
                        )
                        # single merged psum->sbuf extraction (DVE only reader)
                        fw = rpool.tile([128, 3 * F], BT, tag="fw", bufs=3)
                        nc.vector.tensor_copy(fw[:], ps_fwt[:])
                        prod = rpool.tile([128, 3 * F], BT, tag="prod", bufs=3)
                        for d in range(3):
                            nc.vector.tensor_mul(
                                prod[:, d * F:(d + 1) * F],
                                fw[:, F:2 * F],
                                vsb[:, ib * 3 * F + d * F: ib * 3 * F + (d + 1) * F],
                            )
                        ua = unitA[:, ib * JB * 3 + 3 * j: ib * JB * 3 + 3 * j + 3]
                        nc.tensor.matmul(
                            dv_ps[:, 3 * j:3 * j + 3], fw[:, 0:F], ua,
                            start=False, stop=False, skip_group_check=True,
                        )
                        acol = adjs[:, ib * JB + j: ib * JB + j + 1]
                        for d in range(3):
                            nc.tensor.matmul(
                                dv_ps[:, 3 * j + d:3 * j + d + 1],
                                prod[:, d * F:(d + 1) * F], acol,
                                start=False,
                                stop=(j == JB - 1 and ib == NIB - 1 and d == 2),
                                skip_group_check=True,
                            )
                        nc.tensor.matmul(
                            dh_pre[:, j:j + 1], fw[:, 2 * F:3 * F], acol,
                            start=False, stop=(j == JB - 1 and ib == NIB - 1),
                            skip_group_check=True,
                        )

                # ---- dv final: add the b6v*vadj bias, ship out --------
                nc.vector.tensor_add(dv_sb[:], dv_ps[:], bv_vadj[:])

                # ---- dh tail: mul by h, transpose, layernorm ----------
                dhm = wpool.tile([F, JB], DT, tag="dhm")
                nc.vector.tensor_mul(dhm[:], dh_pre[:], hTjf[:])
                ps_dt = mps1.tile([JB, F], DT, tag="gm", bufs=1)
                nc.tensor.transpose(ps_dt[:], dhm[:], ident[:])
                sum_col = wpool.tile([JB, 1], DT, tag="sum_col")
                nc.vector.tensor_reduce(sum_col[:], ps_dt[:], axis=AX.X, op=ALU.add)
                mean_col = wpool.tile([JB, 1], DT, tag="mean_col")
                nc.vector.tensor_scalar_mul(mean_col[:], sum_col[:], 1.0 / F)
                cent = wpool.tile([JB, F], DT, tag="cent")
                nc.vector.tensor_scalar_sub(cent[:], ps_dt[:], mean_col[:])
                sq2 = wpool.tile([JB, F], DT, tag="sq2")
                ssq = wpool.tile([JB, 1], DT, tag="ssq")
                nc.scalar.activation(sq2[:], cent[:], AF.Square, accum_out=ssq[:])
                std = wpool.tile([JB, 1], DT, tag="std")
                nc.scalar.activation(std[:], ssq[:], AF.Sqrt, bias=eps_col[:], scale=1.0 / F)
                rstd = wpool.tile([JB, 1], DT, tag="rstd")
                nc.vector.reciprocal(rstd[:], std[:])
                dh1 = wpool.tile([JB, F], DT, tag="dh1")
                nc.vector.scalar_tensor_tensor(
                    dh1[:], cent[:], rstd[:], gamma_bc[:], op0=ALU.mult, op1=ALU.mult
                )
                dh_fin = wpool.tile([JB, F], DT, tag="dh_fin")
                nc.vector.tensor_add(dh_fin[:], dh1[:], beta_bc[:])
                nc.gpsimd.dma_start(out=dh_out_d[:], in_=dh_fin[:])
                nc.gpsimd.dma_start(out=dv_out_d[:], in_=dv_sb[:])

    _split_multiwaits(nc)
    return nc


def _split_multiwaits(nc: bass.Bass):
    """Move all but one sync-wait of each instruction onto same-engine NOPs
    inserted immediately before it (walrus allows one wait per hw inst)."""
    f = nc.m.functions[0]
    for blk in f.blocks:
        il = blk.instructions  # live list backing the block
        idx = 0
        while idx < len(il):
            inst = il[idx]
            si = inst.sync_info
            if si is not None and si.on_wait and len(si.on_wait) > 1:
                waits = list(si.on_wait)
                extra, keep = waits[:-1], waits[-1:]
                nops = []
                for w in extra:
                    bi = nc.engines[inst.engine].nop()
                    nop = bi.ins
                    cur_list = nc.cur_bb.bb.instructions
                    assert cur_list and cur_list[-1].name == nop.name
                    cur_list.pop()
                    nop.sync_info = mybir.SyncInfo(on_wait=[w], on_update=[])
                    nop.bass_nofuse = True
                    nops.append(nop)
                inst.sync_info = mybir.SyncInfo(on_wait=keep, on_update=list(si.on_update))
                for k, nop in enumerate(nops):
                    il.insert(idx + k, nop)
                idx += len(nops)
            idx += 1


_CACHE: dict = {}


def _get_program():
    if "nc" not in _CACHE:
        _CACHE["nc"] = build_program()
    return _CACHE["nc"]


def kernel(h, v, adj, xyz, W1, b1, W2, b2, W3, b3, W4, b4, W5, b5, W6, b6,
           gamma, beta, offset):
    h = np.asarray(h, np.float32)
    v = np.asarray(v, np.float32)
    adj = np.asarray(adj, np.float32)
    xyz = np.asarray(xyz, np.float32)
    f32 = lambda x: np.ascontiguousarray(np.asarray(x, np.float32))
    b16 = lambda x: np.ascontiguousarray(np.asarray(x, np.float32).astype(BF_NP))

    # W6 column permutation: output channel (f, c) lives at column f*3+c.
    W6 = np.asarray(W6, np.float32)
    b6 = np.asarray(b6, np.float32)
    perm = np.concatenate([np.arange(c, 3 * F, 3) for c in range(3)])
    W6p = W6[:, perm]
    b6r, b6v, b6h = b6[0::3], b6[1::3], b6[2::3]

    shared = {
        "W1": b16(W1), "W2": b16(W2), "W3": b16(W3), "W4": b16(W4), "W5": b16(W5),
        "W6p": b16(W6p),
        "b1": f32(b1).reshape(F, 1), "b2": f32(b2).reshape(F, 1),
        "b3": f32(b3).reshape(F, 1), "b4": f32(b4).reshape(F, 1),
        "b5": f32(b5).reshape(F, 1),
        "b6r": b16(b6r).reshape(F)[None, :], "b6v": f32(b6v).reshape(F, 1),
        "b6h": f32(b6h).reshape(1, F),
        "gamma": f32(gamma).reshape(1, F), "beta": f32(beta).reshape(1, F),
        "offr": f32(offset).reshape(1, F),
        "ident": np.ascontiguousarray(np.eye(128, dtype=np.float32)),
        "identb": np.ascontiguousarray(np.eye(128, dtype=np.float32).astype(BF_NP)),
    }

    in_maps = []
    for c in range(NC):
        b, q = divmod(c, NC // B)
        js = q * JB
        m = dict(shared)
        m["hT"] = b16(h[b].T)
        m["hTj"] = b16(h[b, js:js + JB, :].T)
        m["hTjf"] = f32(h[b, js:js + JB, :].T)
        m["xyzT"] = f32(xyz[b].T)
        m["xyzTj"] = f32(xyz[b, js:js + JB, :].T)
        m["xyzjd"] = f32(xyz[b, js:js + JB, :].reshape(1, JB * 3))
        m["xyzr"] = f32(xyz[b])
        m["adjp"] = b16(adj[b][:, js:js + JB].reshape(NIB, 128, JB))
        m["adjf"] = f32(adj[b][:, js:js + JB].reshape(NIB, 128, JB))
        m["vp"] = b16(v[b].transpose(0, 2, 1).reshape(NIB, 128, 3, F))
        in_maps.append(m)

    nc = _get_program()
    _CACHE["in_maps"] = in_maps
    res = run_bass_kernel_spmd(nc, in_maps, core_ids=list(range(NC))).results

    dh = np.empty((B, N, F), np.float32)
    dv = np.empty((B, N, F, 3), np.float32)
    for c in range(NC):
        b, q = divmod(c, NC // B)
        js = q * JB
        dh[b, js:js + JB] = res[c]["dh_out"]
        dv[b, js:js + JB] = res[c]["dv_out"].reshape(F, JB, 3).transpose(1, 0, 2)
    return dh, dv
